# revision 9
# baseline (speedup 1.0000x reference)
"""Trainium2 kernel for nn_DDApprox: batched DDOpt (Wilson-Dirac D^dag D) applied
to a fixed basis, over B=256 gauge configs.

Key observation: for each gauge config b, DDOpt is a linear operator on C^128
(L*L*2 = 128 spinor components). With the basis as rows Psi (K,128):

    out_b = Psi @ M_b,   M_b = D_b^T G5 D_b^T G5 = A_b @ A_b,  A_b = D_b^T * g5

D_b is a 5-point stencil matrix built directly from the U(1) links on the host
(cheap: ~9 nonzeros/row). The device runs a batched real matmul in block form
(contract 256 = [re|im] components) with output columns interleaved (re,im).

v2 changes vs baseline:
  - int8 output with a single host-calibrated global scale (dequantized on the
    host): halves the dominant output DMA stream. Device casts psum fp32 ->
    int8 (round-to-nearest-even, saturating) in the same copy op.
  - both block rows of R ([Mr|Mi] and [-Mi|Mr]) are built on the host and
    shipped (bf16): no on-device build ops, Vector/Scalar/GpSimd only drain
    psum. Copies are spread over all three engines.
  - all loads issued upfront on sync (R fully resident: 4x8KB/partition),
    warm-up matmuls on a zeroed dummy tile ramp the PE clock during the load.

Sharding: data-parallel over B across 8 cores (32 configs each); every core
holds the full (small) basis.
"""
import numpy as np
import ml_dtypes

import concourse.bass as bass
import concourse.mybir as mybir
import concourse.tile as tile
from concourse import bacc
from concourse.bass_utils import run_bass_kernel_spmd

N_CORES = 8
B, K, L = 256, 512, 8
KAPPA = 0.276
B_PER_CORE = B // N_CORES
N_PAIR = B_PER_CORE // 2   # 2 configs share a matmul free dim
N_GRP = N_PAIR // 4        # 4 pairs share one R load

_G0 = np.array([[0, 1], [1, 0]], np.complex64)
_G1 = np.array([[0, -1j], [1j, 0]], np.complex64)


def _build_M(u1_real, u1_imag):
    """Dense DDOpt^T matrices: M_b such that out_b = Psi @ M_b."""
    U = (u1_real + 1j * u1_imag).astype(np.complex64)  # (B,2,L,L)
    Bn = U.shape[0]
    n = 2 * L * L
    D = np.zeros((Bn, n, n), np.complex64)
    idx = np.arange(n)
    D[:, idx, idx] = 1.0

    x, y = np.meshgrid(np.arange(L), np.arange(L), indexing="ij")
    site = (x * L + y).ravel()
    xp = ((x + 1) % L * L + y).ravel()
    xm = ((x - 1) % L * L + y).ravel()
    yp = (x * L + (y + 1) % L).ravel()
    ym = (x * L + (y - 1) % L).ravel()
    s = np.arange(2)

    def scatter(nbr_site, P, coeff):
        rows = np.broadcast_to(site[:, None, None] * 2 + s[None, :, None], (64, 2, 2)).ravel()
        cols = np.broadcast_to(nbr_site[:, None, None] * 2 + s[None, None, :], (64, 2, 2)).ravel()
        vals = (coeff[:, :, None, None] * P[None, None, :, :]).reshape(Bn, -1)
        D[:, rows, cols] += -KAPPA * vals

    U0 = U[:, 0].reshape(Bn, -1)
    U1 = U[:, 1].reshape(Bn, -1)
    I2 = np.eye(2, dtype=np.complex64)
    scatter(xp, I2 - _G0, U0)
    scatter(xm, I2 + _G0, np.conj(U0[:, xm]))
    scatter(yp, I2 - _G1, U1)
    scatter(ym, I2 + _G1, np.conj(U1[:, ym]))

    g5v = np.tile(np.array([1.0, -1.0], np.float32), L * L)
    A = D.transpose(0, 2, 1) * g5v[None, None, :]
    return (A @ A).astype(np.complex64)


def _build_device_inputs(u1_real, u1_imag, basis_real, basis_imag):
    """psit (128,2,K) bf16, R (B/8, 128, 2, 4, 512) bf16 (both block rows,
    grouped 4 pairs/load), and the int8 output scale s."""
    M = _build_M(u1_real, u1_imag)
    Bn = M.shape[0]
    Mr, Mi = M.real.astype(np.float32), M.imag.astype(np.float32)
    # Full block matrix, contract rows c=0: [Mr | Mi], c=1: [-Mi | Mr], with
    # columns (re,im)-interleaved so the result views as complex64.
    Rf = np.empty((Bn, 2, 128, 256), np.float32)
    Rf[:, 0, :, 0::2] = Mr
    Rf[:, 0, :, 1::2] = Mi
    Rf[:, 1, :, 0::2] = -Mi
    Rf[:, 1, :, 1::2] = Mr

    PsiT = np.concatenate(
        [basis_real.reshape(K, 128).T, basis_imag.reshape(K, 128).T], axis=0
    ).astype(np.float32)  # (256, K)

    # int8 scale: per-b max col 2-norm is exact/cheap; the universal
    # max/colnorm factor is calibrated on 16 sampled configs (+8% margin).
    colmax = np.sqrt((Rf ** 2).sum(axis=(1, 2))).max(axis=1)  # (B,)
    P3 = PsiT.reshape(2, 128, K)
    samp = np.arange(0, Bn, max(1, Bn // 16))
    c_factor = max(
        np.abs(np.einsum('cik,cij->kj', P3, Rf[b], optimize=True)).max() / colmax[b]
        for b in samp
    )
    bound = colmax.max() * c_factor * 1.08
    s = np.float32(127.0 / bound)

    # psit_dev (128,4,2,128): [p,kt,c,j] = PsiT[c*128+p, j*4+kt]
    # (k interleaved so psum tile kt holds k = p*4+kt -> out rows land in
    #  natural k order without any host-side gather; kt-major so the kt0
    #  slab is one small contiguous load)
    PsiT_perm = PsiT.reshape(256, 128, 4).transpose(0, 2, 1)   # [r, kt, j]
    psit_dev = np.ascontiguousarray(
        PsiT_perm.reshape(2, 128, 4, 128).transpose(1, 2, 0, 3))

    # R device layout [G, 128i, 2c, 4q, 512]: [g, i, c, q, cfg*256+col] =
    # Rf[8g+2q+cfg, c, i, col]  (2 configs share the matmul free dim)
    Rg = Rf.reshape(Bn // 8, 4, 2, 2, 128, 256)        # [g, q, cfg, c, i, col]
    Rg = Rg.transpose(0, 4, 3, 1, 2, 5)                # [g, i, c, q, cfg, col]
    Rg = np.ascontiguousarray(Rg).reshape(Bn // 8, 128, 2, 4, 512)
    return (
        psit_dev.astype(ml_dtypes.bfloat16),
        Rg.astype(ml_dtypes.bfloat16),
        s,
    )


def _build_nc(n_b, s, mm_dt=mybir.dt.bfloat16):
    """Per-core kernel: out[b] (K,256) = PsiT.T (K,256c) @ R[b] (256c,256),
    cast to int8 with scale s on the psum drain."""
    nc = bacc.Bacc(None, target_bir_lowering=False)
    n_grp = n_b // 8
    f32 = mybir.dt.float32
    i8 = mybir.dt.int8
    psit = nc.dram_tensor("psit", [128, 4, 2, 128], mm_dt, kind="ExternalInput")
    r = nc.dram_tensor("r", [n_grp, 128, 2, 4, 512], mm_dt, kind="ExternalInput")
    # out[g, p, kt, q, cfg*256+col]: config b = 8g+2q+cfg, row k = 4p+kt
    out = nc.dram_tensor("out", [n_grp, 128, 4, 4, 512], i8, kind="ExternalOutput")

    with tile.TileContext(nc) as tc:
        with (
            tc.tile_pool(name="singles", bufs=1) as singles,
            tc.tile_pool(name="rpool", bufs=n_grp) as rpool,
            tc.tile_pool(name="opool", bufs=2) as opool,
            tc.tile_pool(name="psum", bufs=4, space="PSUM") as psum_pool,
        ):
            psit_sb = singles.tile([128, 4, 2, 128], mm_dt)
            warm = singles.tile([128, 640], mm_dt)
            r_sb = [rpool.tile([128, 2, 4, 512], mm_dt, name="r_sb") for _ in range(n_grp)]
            # all loads upfront on sync, in consumption order: psit kt0 slab,
            # then group 0 split per q-pair (earliest possible first matmul),
            # then the rest
            nc.sync.dma_start(out=psit_sb[:, 0], in_=psit[:, 0])
            nc.sync.dma_start(out=r_sb[0][:, :, 0:2], in_=r[0][:, :, 0:2])
            nc.sync.dma_start(out=psit_sb[:, 1:4], in_=psit[:, 1:4])
            nc.sync.dma_start(out=r_sb[0][:, :, 2:4], in_=r[0][:, :, 2:4])
            for g in range(1, n_grp):
                nc.sync.dma_start(out=r_sb[g][:], in_=r[g])

            # PE clock warm-up on a zeroed tile while loads stream (ends right
            # as the first data lands)
            nc.gpsimd.memset(warm[:], 0.0)
            for i in range(4):
                wps = psum_pool.tile([128, 2, 512], f32, name="ps")
                nc.tensor.matmul(
                    wps[:, i % 2, :], warm[:, 0:128], warm[:, 128:640],
                    start=True, stop=True,
                )

            # GPSIMD cannot access PSUM; only DVE + ACT drain it, alternating
            # per half-kt (2 psum banks each) so the recycle chain has slack.
            # Matmuls run half-major (q-pair completes before the next pair
            # starts) so each drain fires as early as possible.
            cp_engines = [nc.vector, nc.scalar]
            st_engines = [nc.sync, nc.gpsimd]
            n_cp = 0
            for g in range(n_grp):
                otile = opool.tile([128, 4, 4, 512], i8, name="o_sb")
                last_g = g == n_grp - 1
                for kt in range(4):
                    lhsT = [psit_sb[:, kt, c, :] for c in range(2)]
                    for h in range(2):
                        ph = psum_pool.tile([128, 2, 512], f32, name="ps")
                        for c in range(2):
                            for qh in range(2):
                                nc.tensor.matmul(
                                    ph[:, qh, :], lhsT[c],
                                    r_sb[g][:, c, 2 * h + qh, :],
                                    start=(c == 0), stop=(c == 1),
                                )
                        eng = cp_engines[n_cp % 2]
                        n_cp += 1
                        dst = otile[:, kt, 2 * h:2 * h + 2]
                        if eng is nc.scalar:
                            eng.mul(dst, ph[:], float(s))
                        else:
                            eng.tensor_scalar_mul(dst, ph[:], float(s))
                        if last_g and kt >= 2:
                            # finest tail: per-half stores at the very end
                            st = st_engines[(2 * kt + h) % 2]
                            st.dma_start(out=out[g][:, kt, 2 * h:2 * h + 2],
                                         in_=dst)
                    if last_g:
                        if kt == 1:
                            st = st_engines[0]
                            st.dma_start(out=out[g][:, 0:2], in_=otile[:, 0:2])
                    elif kt % 2 == 1:
                        st = st_engines[(2 * g + kt // 2) % 2]
                        st.dma_start(
                            out=out[g][:, kt - 1:kt + 1],
                            in_=otile[:, kt - 1:kt + 1],
                        )
    nc.compile()
    return nc


def kernel(u1_real, u1_imag, basis_real, basis_imag, _want_results_obj=False, _trace=False):
    u1_real = np.asarray(u1_real, np.float32)
    u1_imag = np.asarray(u1_imag, np.float32)
    basis_real = np.asarray(basis_real, np.float32)
    basis_imag = np.asarray(basis_imag, np.float32)

    PsiT, R, s = _build_device_inputs(u1_real, u1_imag, basis_real, basis_imag)
    nc = _build_nc(B_PER_CORE, s)
    in_maps = [
        {"psit": PsiT, "r": np.ascontiguousarray(R[i * N_GRP:(i + 1) * N_GRP])}
        for i in range(N_CORES)
    ]
    res = run_bass_kernel_spmd(nc, in_maps, core_ids=list(range(N_CORES)), trace=_trace)
    full = np.concatenate(
        [np.asarray(res.results[i]["out"]) for i in range(N_CORES)], axis=0
    )  # (32, 128, 4, 4, 512) int8: [g, p, kt, q, cfg*256+col], b=8g+2q+cfg, k=4p+kt
    full = full.transpose(0, 3, 1, 2, 4)          # (g, q, p, kt, 512)
    full = full.reshape(32, 4, K, 2, 256)         # (g, q, k, cfg, col)
    full = full.transpose(0, 1, 3, 2, 4)          # (g, q, cfg, k, col)
    deq = np.ascontiguousarray(full).reshape(B, K, 256).astype(np.float32)
    deq *= np.float32(1.0) / s
    out = deq.view(np.complex64)  # (B,K,128)
    if _want_results_obj:
        return out, res
    return out


# revision 10
# speedup vs baseline: 1.0358x; 1.0358x over previous
"""Trainium2 kernel for nn_DDApprox: batched DDOpt (Wilson-Dirac D^dag D) applied
to a fixed basis, over B=256 gauge configs.

Key observation: for each gauge config b, DDOpt is a linear operator on C^128
(L*L*2 = 128 spinor components). With the basis as rows Psi (K,128):

    out_b = Psi @ M_b,   M_b = D_b^T G5 D_b^T G5 = A_b @ A_b,  A_b = D_b^T * g5

D_b is a 5-point stencil matrix built directly from the U(1) links on the host
(cheap: ~9 nonzeros/row). The device runs a batched real matmul in block form
(contract 256 = [re|im] components) with output columns interleaved (re,im).

v2 changes vs baseline:
  - int8 output with a single host-calibrated global scale (dequantized on the
    host): halves the dominant output DMA stream. Device casts psum fp32 ->
    int8 (round-to-nearest-even, saturating) in the same copy op.
  - both block rows of R ([Mr|Mi] and [-Mi|Mr]) are built on the host and
    shipped (bf16): no on-device build ops, Vector/Scalar/GpSimd only drain
    psum. Copies are spread over all three engines.
  - all loads issued upfront on sync (R fully resident: 4x8KB/partition),
    warm-up matmuls on a zeroed dummy tile ramp the PE clock during the load.

Sharding: data-parallel over B across 8 cores (32 configs each); every core
holds the full (small) basis.
"""
import numpy as np
import ml_dtypes

import concourse.bass as bass
import concourse.mybir as mybir
import concourse.tile as tile
from concourse import bacc
from concourse.bass_utils import run_bass_kernel_spmd

N_CORES = 8
B, K, L = 256, 512, 8
KAPPA = 0.276
B_PER_CORE = B // N_CORES
N_PAIR = B_PER_CORE // 2   # 2 configs share a matmul free dim
N_GRP = N_PAIR // 4        # 4 pairs share one R load

_G0 = np.array([[0, 1], [1, 0]], np.complex64)
_G1 = np.array([[0, -1j], [1j, 0]], np.complex64)


def _build_M(u1_real, u1_imag):
    """Dense DDOpt^T matrices: M_b such that out_b = Psi @ M_b."""
    U = (u1_real + 1j * u1_imag).astype(np.complex64)  # (B,2,L,L)
    Bn = U.shape[0]
    n = 2 * L * L
    D = np.zeros((Bn, n, n), np.complex64)
    idx = np.arange(n)
    D[:, idx, idx] = 1.0

    x, y = np.meshgrid(np.arange(L), np.arange(L), indexing="ij")
    site = (x * L + y).ravel()
    xp = ((x + 1) % L * L + y).ravel()
    xm = ((x - 1) % L * L + y).ravel()
    yp = (x * L + (y + 1) % L).ravel()
    ym = (x * L + (y - 1) % L).ravel()
    s = np.arange(2)

    def scatter(nbr_site, P, coeff):
        rows = np.broadcast_to(site[:, None, None] * 2 + s[None, :, None], (64, 2, 2)).ravel()
        cols = np.broadcast_to(nbr_site[:, None, None] * 2 + s[None, None, :], (64, 2, 2)).ravel()
        vals = (coeff[:, :, None, None] * P[None, None, :, :]).reshape(Bn, -1)
        D[:, rows, cols] += -KAPPA * vals

    U0 = U[:, 0].reshape(Bn, -1)
    U1 = U[:, 1].reshape(Bn, -1)
    I2 = np.eye(2, dtype=np.complex64)
    scatter(xp, I2 - _G0, U0)
    scatter(xm, I2 + _G0, np.conj(U0[:, xm]))
    scatter(yp, I2 - _G1, U1)
    scatter(ym, I2 + _G1, np.conj(U1[:, ym]))

    g5v = np.tile(np.array([1.0, -1.0], np.float32), L * L)
    A = D.transpose(0, 2, 1) * g5v[None, None, :]
    return (A @ A).astype(np.complex64)


def _build_device_inputs(u1_real, u1_imag, basis_real, basis_imag):
    """psit (128,2,K) bf16, R (B/8, 128, 2, 4, 512) bf16 (both block rows,
    grouped 4 pairs/load), and the int8 output scale s."""
    M = _build_M(u1_real, u1_imag)
    Bn = M.shape[0]
    Mr, Mi = M.real.astype(np.float32), M.imag.astype(np.float32)
    # Full block matrix, contract rows c=0: [Mr | Mi], c=1: [-Mi | Mr], with
    # columns (re,im)-interleaved so the result views as complex64.
    Rf = np.empty((Bn, 2, 128, 256), np.float32)
    Rf[:, 0, :, 0::2] = Mr
    Rf[:, 0, :, 1::2] = Mi
    Rf[:, 1, :, 0::2] = -Mi
    Rf[:, 1, :, 1::2] = Mr

    PsiT = np.concatenate(
        [basis_real.reshape(K, 128).T, basis_imag.reshape(K, 128).T], axis=0
    ).astype(np.float32)  # (256, K)

    # int8 scale: per-b max col 2-norm is exact/cheap; the universal
    # max/colnorm factor is calibrated on 16 sampled configs (+8% margin).
    colmax = np.sqrt((Rf ** 2).sum(axis=(1, 2))).max(axis=1)  # (B,)
    P3 = PsiT.reshape(2, 128, K)
    samp = np.arange(0, Bn, max(1, Bn // 16))
    c_factor = max(
        np.abs(np.einsum('cik,cij->kj', P3, Rf[b], optimize=True)).max() / colmax[b]
        for b in samp
    )
    bound = colmax.max() * c_factor * 1.08
    s = np.float32(127.0 / bound)

    # psit_dev (128,4,2,128): [p,kt,c,j] = PsiT[c*128+p, j*4+kt]
    # (k interleaved so psum tile kt holds k = p*4+kt -> out rows land in
    #  natural k order without any host-side gather; kt-major so the kt0
    #  slab is one small contiguous load)
    PsiT_perm = PsiT.reshape(256, 128, 4).transpose(0, 2, 1)   # [r, kt, j]
    psit_dev = np.ascontiguousarray(
        PsiT_perm.reshape(2, 128, 4, 128).transpose(1, 2, 0, 3))

    # R device layout [G, 128i, 2c, 4q, 512]: [g, i, c, q, cfg*256+col] =
    # Rf[8g+2q+cfg, c, i, col]  (2 configs share the matmul free dim)
    Rg = Rf.reshape(Bn // 8, 4, 2, 2, 128, 256)        # [g, q, cfg, c, i, col]
    Rg = Rg.transpose(0, 4, 3, 1, 2, 5)                # [g, i, c, q, cfg, col]
    Rg = np.ascontiguousarray(Rg).reshape(Bn // 8, 128, 2, 4, 512)
    return (
        psit_dev.astype(ml_dtypes.bfloat16),
        Rg.astype(ml_dtypes.bfloat16),
        s,
    )


def _build_nc(n_b, s, mm_dt=mybir.dt.bfloat16):
    """Per-core kernel: out[b] (K,256) = PsiT.T (K,256c) @ R[b] (256c,256),
    cast to int8 with scale s on the psum drain."""
    nc = bacc.Bacc(None, target_bir_lowering=False)
    n_grp = n_b // 8
    f32 = mybir.dt.float32
    i8 = mybir.dt.int8
    psit = nc.dram_tensor("psit", [128, 4, 2, 128], mm_dt, kind="ExternalInput")
    r = nc.dram_tensor("r", [n_grp, 128, 2, 4, 512], mm_dt, kind="ExternalInput")
    # out[g, p, kt, q, cfg*256+col]: config b = 8g+2q+cfg, row k = 4p+kt
    out = nc.dram_tensor("out", [n_grp, 128, 4, 4, 512], i8, kind="ExternalOutput")

    with tile.TileContext(nc) as tc:
        with (
            tc.tile_pool(name="singles", bufs=1) as singles,
            tc.tile_pool(name="rpool", bufs=n_grp) as rpool,
            tc.tile_pool(name="opool", bufs=2) as opool,
            tc.tile_pool(name="psum", bufs=4, space="PSUM") as psum_pool,
        ):
            psit_sb = singles.tile([128, 4, 2, 128], mm_dt)
            warm = singles.tile([128, 640], mm_dt)
            r_sb = [rpool.tile([128, 2, 4, 512], mm_dt, name="r_sb") for _ in range(n_grp)]
            # loads upfront on two queues: psit on gpsimd, R on sync, group 0
            # split per q-pair (earliest possible first matmul)
            nc.gpsimd.memset(warm[:], 0.0)
            nc.gpsimd.dma_start(out=psit_sb[:, 0], in_=psit[:, 0])
            nc.gpsimd.dma_start(out=psit_sb[:, 1:4], in_=psit[:, 1:4])
            nc.sync.dma_start(out=r_sb[0][:, :, 0:2], in_=r[0][:, :, 0:2])
            nc.sync.dma_start(out=r_sb[0][:, :, 2:4], in_=r[0][:, :, 2:4])
            for g in range(1, n_grp):
                nc.sync.dma_start(out=r_sb[g][:], in_=r[g])

            # PE clock warm-up on a zeroed tile while loads stream (ends right
            # as the first data lands; idle PE drops back to a cold clock)
            for i in range(7):
                wps = psum_pool.tile([128, 2, 512], f32, name="ps")
                nc.tensor.matmul(
                    wps[:, i % 2, :], warm[:, 0:128], warm[:, 128:640],
                    start=True, stop=True,
                )

            # GPSIMD cannot access PSUM; only DVE + ACT drain it, alternating
            # per half-kt (2 psum banks each) so the recycle chain has slack.
            # Matmuls run half-major (q-pair completes before the next pair
            # starts) so each drain fires as early as possible.
            cp_engines = [nc.vector, nc.scalar]
            st_engines = [nc.sync, nc.gpsimd]
            n_cp = 0
            for g in range(n_grp):
                otile = opool.tile([128, 4, 4, 512], i8, name="o_sb")
                last_g = g == n_grp - 1
                for kt in range(4):
                    lhsT = [psit_sb[:, kt, c, :] for c in range(2)]
                    for h in range(2):
                        ph = psum_pool.tile([128, 2, 512], f32, name="ps")
                        for c in range(2):
                            for qh in range(2):
                                nc.tensor.matmul(
                                    ph[:, qh, :], lhsT[c],
                                    r_sb[g][:, c, 2 * h + qh, :],
                                    start=(c == 0), stop=(c == 1),
                                )
                        eng = cp_engines[n_cp % 2]
                        n_cp += 1
                        dst = otile[:, kt, 2 * h:2 * h + 2]
                        if eng is nc.scalar:
                            eng.mul(dst, ph[:], float(s))
                        else:
                            eng.tensor_scalar_mul(dst, ph[:], float(s))
                        if last_g and kt >= 2:
                            # finest tail: per-half stores at the very end
                            st = st_engines[(2 * kt + h) % 2]
                            st.dma_start(out=out[g][:, kt, 2 * h:2 * h + 2],
                                         in_=dst)
                    if last_g:
                        if kt == 1:
                            st = st_engines[0]
                            st.dma_start(out=out[g][:, 0:2], in_=otile[:, 0:2])
                    elif kt % 2 == 1:
                        st = st_engines[(2 * g + kt // 2) % 2]
                        st.dma_start(
                            out=out[g][:, kt - 1:kt + 1],
                            in_=otile[:, kt - 1:kt + 1],
                        )
    nc.compile()
    return nc


def kernel(u1_real, u1_imag, basis_real, basis_imag, _want_results_obj=False, _trace=False):
    u1_real = np.asarray(u1_real, np.float32)
    u1_imag = np.asarray(u1_imag, np.float32)
    basis_real = np.asarray(basis_real, np.float32)
    basis_imag = np.asarray(basis_imag, np.float32)

    PsiT, R, s = _build_device_inputs(u1_real, u1_imag, basis_real, basis_imag)
    nc = _build_nc(B_PER_CORE, s)
    in_maps = [
        {"psit": PsiT, "r": np.ascontiguousarray(R[i * N_GRP:(i + 1) * N_GRP])}
        for i in range(N_CORES)
    ]
    res = run_bass_kernel_spmd(nc, in_maps, core_ids=list(range(N_CORES)), trace=_trace)
    full = np.concatenate(
        [np.asarray(res.results[i]["out"]) for i in range(N_CORES)], axis=0
    )  # (32, 128, 4, 4, 512) int8: [g, p, kt, q, cfg*256+col], b=8g+2q+cfg, k=4p+kt
    full = full.transpose(0, 3, 1, 2, 4)          # (g, q, p, kt, 512)
    full = full.reshape(32, 4, K, 2, 256)         # (g, q, k, cfg, col)
    full = full.transpose(0, 1, 3, 2, 4)          # (g, q, cfg, k, col)
    deq = np.ascontiguousarray(full).reshape(B, K, 256).astype(np.float32)
    deq *= np.float32(1.0) / s
    out = deq.view(np.complex64)  # (B,K,128)
    if _want_results_obj:
        return out, res
    return out
